# revision 1
# baseline (speedup 1.0000x reference)
"""Trainium2 Bass kernel for the DeepHOTCO Grossberg shunting ODE problem.

Strategy: pure data parallel over the agent batch (32768 agents -> 8 cores x
4096). Each core holds its agents as [128 partitions x 32 groups]; the
per-agent 17x17 matvecs run on the Vector engine as a broadcast multiply
followed by a grouped reduce; the elementwise shunting/Euler update is a
fused chain of DVE ops with sigmoids on the Scalar engine. The 127-step loop
is fully unrolled (no Tile back-edge barriers).
"""

import os
import sys


def _ensure_env():
    for p in (
        "/root/.axon_site",
        "/root/.axon_site/_ro/trn_rl_repo",
        "/root/.axon_site/_ro/pypackages",
        "/opt/trn_rl_repo",
    ):
        if os.path.isdir(p) and p not in sys.path:
            sys.path.append(p)
    # If the sitecustomize-driven axon boot never ran (e.g. PYTHONPATH was
    # not set for this process), replicate it. sitecustomize imports
    # trn_agent_boot, so its presence in sys.modules means boot already ran.
    if "trn_agent_boot.trn_boot" not in sys.modules and "jax" not in sys.modules:
        os.environ.setdefault("TRN_TERMINAL_POOL_IPS", "local")
        os.environ.setdefault("AXON_POOL_SVC_OVERRIDE", "127.0.0.1")
        os.environ.setdefault("AXON_LOOPBACK_RELAY", "1")
        try:
            from trn_agent_boot.trn_boot import boot

            boot(
                os.environ.get(
                    "TRN_TERMINAL_PRECOMPUTED_JSON",
                    "/root/.axon_site/_trn_precomputed.json",
                ),
                "/opt/axon/libaxon_pjrt.so",
            )
        except Exception:
            pass


_ensure_env()

import numpy as np  # noqa: E402

import concourse.bass as bass  # noqa: E402
import concourse.tile as tile  # noqa: E402
from concourse import mybir  # noqa: E402
from concourse.bass_utils import run_bass_kernel_spmd  # noqa: E402
from concourse.vector_clock import ScopedClock  # noqa: E402

# ---------------------------------------------------------------------------
# Workaround: walrus in this container only accepts a single sync-wait on the
# CTRL(Drain) instruction Tile emits at kernel tail. Split the accumulated
# waits across a chain of single-wait drains.
_MAX_DRAIN_WAITS = 1


def _patched_drain_and_barrier(self, tick_clock, wait_clock):
    drain_inst = self.nc.sync.drain()
    wait_clock.add_sem_waits(
        drain_inst.ins, ScopedClock({None: tick_clock.global_clock})
    )
    si = drain_inst.ins.sync_info
    if si is not None and si.on_wait and len(si.on_wait) > _MAX_DRAIN_WAITS:
        waits = list(si.on_wait)
        si.on_wait.clear()
        si.on_wait.extend(waits[:_MAX_DRAIN_WAITS])
        rest = waits[_MAX_DRAIN_WAITS:]
        for i in range(0, len(rest), _MAX_DRAIN_WAITS):
            extra = self.nc.sync.drain()
            esi = extra.ins.sync_info
            if esi is None:
                extra.ins.sync_info = mybir.SyncInfo(on_wait=[], on_update=[])
                esi = extra.ins.sync_info
            esi.on_wait.extend(rest[i : i + _MAX_DRAIN_WAITS])
    self.nc.all_engine_barrier()
    assert self.sems is not None
    popped = self.nc._tile_sem_poison_stack.pop()
    assert popped is self._sem_poison
    self.nc.clear_and_free_semaphores(list(self.sems.allocated().values()))
    self.nc.all_engine_barrier()


tile.TileContext._drain_and_barrier = _patched_drain_and_barrier

_waitsplit_counter = [0]


def _split_excess_waits(nc, max_waits=_MAX_DRAIN_WAITS):
    """This container's walrus accepts only one sync-wait per instruction;
    move excess waits onto same-engine NOPs inserted just before."""
    for func in nc.m.functions:
        for bb in func.blocks:
            insts = list(bb.instructions)
            needs = any(
                getattr(i, "sync_info", None) is not None
                and i.sync_info.on_wait
                and len(i.sync_info.on_wait) > max_waits
                for i in insts
            )
            if not needs:
                continue
            new_list = []
            for inst in insts:
                si = getattr(inst, "sync_info", None)
                if si is not None and si.on_wait and len(si.on_wait) > max_waits:
                    waits = list(si.on_wait)
                    del si.on_wait[max_waits:]
                    rest = waits[max_waits:]
                    for k in range(0, len(rest), max_waits):
                        _waitsplit_counter[0] += 1
                        nop = mybir.InstNoOp(
                            name=f"I-waitsplit-{_waitsplit_counter[0]}",
                            engine=inst.engine,
                            sync_info=mybir.SyncInfo(
                                on_wait=list(rest[k : k + max_waits]),
                                on_update=[],
                            ),
                        )
                        nc.register_instruction(nop)
                        new_list.append(nop)
                new_list.append(inst)
            bb.instructions[:] = new_list

# ---------------------------------------------------------------------------
# Problem constants (hardcoded per spec.json)
NCORES = 8
BATCH = 32768
P = 128          # SBUF partitions = agents per partition-block
G = 32           # agent groups along the free dim (P * G = agents per core)
N = 17           # nodes per agent
T = 128          # trajectory length (126 Euler steps + initial state)
NSTEP = T - 1
BLOC = P * G     # agents per core

TAU = 0.8
DECAY = 0.15
C_FLOOR = 0.1
LAT_INHIB = 3.0
DIV_SIGMA = 0.3
ALPHA = 1.5
BETA = 0.75
DT = 0.05
DT_TAU = DT / TAU  # 0.0625 exactly

F32 = mybir.dt.float32
AX = mybir.AxisListType
OP = mybir.AluOpType
ACTF = mybir.ActivationFunctionType


BF16 = mybir.dt.bfloat16
F16 = mybir.dt.float16
M = 2 * N  # 34 rows: [0:17] = W_pos output, [17:34] = W_neg output


def build_program(n_steps=NSTEP):
    """Combined fp16 W (pos||neg, feasibility pre-folded into pos action
    rows on host), 2x-mode fp16 broadcast multiply, pairwise-fold tree
    reduction (2x TT adds) instead of the 1x-only tensor_reduce."""
    nc = bass.Bass("TRN2", target_bir_lowering=False, debug=False,
                   num_devices=NCORES)

    x_state = nc.dram_tensor("state0", [BLOC, N], F32, kind="ExternalInput")
    # host-packed: wmain = W[:, :, 0:16] fp16, w16 = W[:, :, 16] fp16
    x_wm = nc.dram_tensor("wmain", [BLOC, M, 16], F16, kind="ExternalInput")
    x_w16 = nc.dram_tensor("w16", [BLOC, M], F16, kind="ExternalInput")
    x_pe = nc.dram_tensor("pert", [BLOC, N], F32, kind="ExternalInput")
    y = nc.dram_tensor("out", [n_steps + 1, BLOC, N], F32,
                       kind="ExternalOutput")

    from contextlib import ExitStack

    with tile.TileContext(nc) as tc, ExitStack() as ctx:
        consts = ctx.enter_context(tc.tile_pool(name="consts", bufs=1))
        states = ctx.enter_context(tc.tile_pool(name="states", bufs=3))
        tmps = ctx.enter_context(tc.tile_pool(name="tmps", bufs=1))
        folds = ctx.enter_context(tc.tile_pool(name="folds", bufs=1))
        small = ctx.enter_context(tc.tile_pool(name="small", bufs=2))

        # ---- constant loads. Small inputs first so the pre-matvec chain
        # of step 0 can run while the big W stream is still in flight.
        pe = consts.tile([P, G, N], F32, tag="pe")
        nc.sync.dma_start(out=pe, in_=x_pe[:].rearrange("(p g) n -> p g n", p=P))
        cur = states.tile([P, G, N], F32, tag="state")
        nc.sync.dma_start(out=cur,
                          in_=x_state[:].rearrange("(p g) n -> p g n", p=P))

        wm = consts.tile([P, G, M, 16], F16, tag="wm")
        wsrc = x_wm[:].rearrange("(p g) m j -> p g m j", p=P)
        nchunk = 8
        step = G // nchunk
        for c in range(nchunk):
            sl = slice(c * step, (c + 1) * step)
            nc.sync.dma_start(out=wm[:, sl], in_=wsrc[:, sl])
        w16 = consts.tile([P, G, M], F16, tag="w16")
        nc.sync.dma_start(out=w16,
                          in_=x_w16[:].rearrange("(p g) m -> p g m", p=P))

        # relu(P[:, :9]) and relu(-P[:, :9]) are loop constants (fp16)
        pp9 = consts.tile([P, G, 9], F16, tag="pp9")
        nc.vector.tensor_scalar_max(out=pp9, in0=pe[:, :, 0:9], scalar1=0.0)
        pm9 = consts.tile([P, G, 9], F16, tag="pm9")
        nc.vector.tensor_scalar(out=pm9, in0=pe[:, :, 0:9], scalar1=-1.0,
                                scalar2=0.0, op0=OP.mult, op1=OP.max)

        # trajectory row 0 = initial state
        nc.sync.dma_start(out=y[:][0].rearrange("(p g) n -> p g n", p=P),
                          in_=cur)
        sbf = states.tile([P, G, 20], F16, tag="sbf")
        nc.vector.tensor_copy(out=sbf[:, :, 0:N], in_=cur)
        nc.vector.tensor_copy(out=sbf[:, :, N:N + 1], in_=cur[:, :, 16:17])

        # ---- Euler loop (fully unrolled) -----------------------------------
        for t in range(n_steps):
            # valence gates + lateral inhibition depend only on cur; emit
            # them first so ScalarE round-trips hide under the matvec.
            ve = small.tile([P, G, 4], F32, tag="ve")
            nc.vector.tensor_add(out=ve, in0=cur[:, :, 13:17],
                                 in1=pe[:, :, 13:17])
            ge = small.tile([P, G, 4], F16, tag="ge")
            nc.scalar.activation(out=ge, in_=ve, func=ACTF.Sigmoid, scale=ALPHA)
            gi = small.tile([P, G, 4], F16, tag="gi")
            nc.scalar.activation(out=gi, in_=ve, func=ACTF.Sigmoid, scale=-BETA)

            osum = small.tile([P, G, 1], F32, tag="osum")
            nc.vector.tensor_reduce(out=osum, in_=cur[:, :, 9:13], axis=AX.X,
                                    op=OP.add)
            osum3 = small.tile([P, G, 1], F32, tag="osum3")
            nc.scalar.activation(out=osum3, in_=osum, func=ACTF.Copy,
                                 bias=DIV_SIGMA)
            den = small.tile([P, G, 4], F32, tag="den")
            nc.vector.scalar_tensor_tensor(
                out=den, in0=cur[:, :, 9:13], scalar=-1.0,
                in1=osum3.broadcast_to([P, G, 4]), op0=OP.mult, op1=OP.add)
            rec = small.tile([P, G, 4], F32, tag="rec")
            nc.vector.reciprocal(out=rec, in_=den)
            lat = small.tile([P, G, 4], F16, tag="lat")
            nc.scalar.activation(out=lat, in_=rec, func=ACTF.Copy,
                                 scale=-DIV_SIGMA * LAT_INHIB,
                                 bias=LAT_INHIB)

            # batched per-agent matvec: fp16 products at 2x, fold tree
            tmp = tmps.tile([P, G, M, 16], F16, tag="tmp")
            nc.vector.tensor_tensor(
                out=tmp, in0=wm,
                in1=sbf[:, :, None, 0:16].broadcast_to([P, G, M, 16]),
                op=OP.mult)
            t16 = folds.tile([P, G, M], F16, tag="t16")
            nc.vector.tensor_tensor(
                out=t16.rearrange("p g (q r) -> p g q r", r=2),
                in0=w16.rearrange("p g (q r) -> p g q r", r=2),
                in1=sbf[:, :, None, 16:18].broadcast_to([P, G, N, 2]),
                op=OP.mult)
            c1 = folds.tile([P, G, M, 8], F16, tag="c1")
            nc.vector.tensor_add(out=c1, in0=tmp[:, :, :, 0:8],
                                 in1=tmp[:, :, :, 8:16])
            c2 = folds.tile([P, G, M, 4], F16, tag="c2")
            nc.vector.tensor_add(out=c2, in0=c1[:, :, :, 0:4],
                                 in1=c1[:, :, :, 4:8])
            c3 = folds.tile([P, G, M, 2], F16, tag="c3")
            nc.vector.tensor_add(out=c3, in0=c2[:, :, :, 0:2],
                                 in1=c2[:, :, :, 2:4])
            c4 = folds.tile([P, G, M], F16, tag="c4")
            nc.vector.tensor_add(out=c4, in0=c3[:, :, :, 0],
                                 in1=c3[:, :, :, 1])
            ei = small.tile([P, G, M], F16, tag="ei")
            nc.vector.tensor_add(out=ei, in0=c4, in1=t16)
            # ei rows [0:17] = W_pos' @ s (feas folded), [17:34] = W_neg @ s

            # gate action rows, then relu everything on ScalarE
            nc.vector.tensor_mul(out=ei[:, :, 9:13], in0=ei[:, :, 9:13],
                                 in1=ge)
            nc.vector.tensor_mul(out=ei[:, :, 26:30], in0=ei[:, :, 26:30],
                                 in1=gi)
            nc.vector.tensor_scalar_max(out=ei, in0=ei, scalar1=0.0)

            # environmental drive on need rows
            nc.vector.tensor_add(out=ei[:, :, 0:9], in0=ei[:, :, 0:9],
                                 in1=pp9)
            nc.vector.tensor_add(out=ei[:, :, 17:26], in0=ei[:, :, 17:26],
                                 in1=pm9)

            # lateral inhibition adds to action rows of I
            nc.vector.tensor_add(out=ei[:, :, 26:30], in0=ei[:, :, 26:30],
                                 in1=lat)

            # new = s*K1 + DT_TAU*(G2 + Pd)
            #   K1 = 1 - DT_TAU*(E + I + DECAY);  G2 = E - 0.1*I
            sei = small.tile([P, G, N], F32, tag="sei")
            nc.vector.tensor_add(out=sei, in0=ei[:, :, 0:17],
                                 in1=ei[:, :, 17:34])
            k1 = small.tile([P, G, N], F32, tag="k1")
            nc.scalar.activation(out=k1, in_=sei, func=ACTF.Copy,
                                 scale=-DT_TAU, bias=1.0 - DT_TAU * DECAY)
            g2 = small.tile([P, G, N], F32, tag="g2")
            nc.vector.scalar_tensor_tensor(out=g2, in0=ei[:, :, 17:34],
                                           scalar=-C_FLOOR,
                                           in1=ei[:, :, 0:17], op0=OP.mult,
                                           op1=OP.add)
            nc.vector.tensor_add(out=g2[:, :, 13:17], in0=g2[:, :, 13:17],
                                 in1=pe[:, :, 13:17])
            t1 = small.tile([P, G, N], F32, tag="t1")
            nc.vector.tensor_mul(out=t1, in0=cur, in1=k1)
            new = states.tile([P, G, N], F32, tag="state")
            nc.vector.scalar_tensor_tensor(out=new, in0=g2, scalar=DT_TAU,
                                           in1=t1, op0=OP.mult, op1=OP.add)
            nc.vector.tensor_scalar(out=new[:, :, 0:13], in0=new[:, :, 0:13],
                                    scalar1=0.0, scalar2=1.0, op0=OP.max,
                                    op1=OP.min)
            nc.vector.tensor_scalar(out=new[:, :, 13:17], in0=new[:, :, 13:17],
                                    scalar1=-1.0, scalar2=1.0, op0=OP.max,
                                    op1=OP.min)

            nc.sync.dma_start(
                out=y[:][t + 1].rearrange("(p g) n -> p g n", p=P), in_=new)
            cur = new
            if t + 1 < n_steps:
                sbf = states.tile([P, G, 20], F16, tag="sbf")
                nc.vector.tensor_copy(out=sbf[:, :, 0:N], in_=cur)
                nc.vector.tensor_copy(out=sbf[:, :, N:N + 1],
                                      in_=cur[:, :, 16:17])

    _split_excess_waits(nc)
    return nc


_cache = {}


def _get_nc():
    if "nc" not in _cache:
        _cache["nc"] = build_program()
    return _cache["nc"]


def make_in_maps(state0, W_pos, W_neg, feasibility, perturbation):
    import ml_dtypes

    state0 = np.asarray(state0, dtype=np.float32)
    W_pos = np.asarray(W_pos, dtype=np.float32)
    W_neg = np.asarray(W_neg, dtype=np.float32)
    feasibility = np.asarray(feasibility, dtype=np.float32)
    perturbation = np.asarray(perturbation, dtype=np.float32)

    # fold feasibility into W_pos action rows (relu(F*x) == F*relu(x), F>=0)
    Wp = W_pos.copy()
    Wp[:, 9:13, :] *= feasibility[:, :, None]
    Wc = np.concatenate([Wp, W_neg], axis=1)  # [B, 34, 17]
    wmain = np.ascontiguousarray(Wc[:, :, 0:16]).astype(np.float16)
    w16 = np.ascontiguousarray(Wc[:, :, 16]).astype(np.float16)

    in_maps = []
    for c in range(NCORES):
        sl = slice(c * BLOC, (c + 1) * BLOC)
        in_maps.append({
            "state0": np.ascontiguousarray(state0[sl]),
            "wmain": np.ascontiguousarray(wmain[sl]),
            "w16": np.ascontiguousarray(w16[sl]),
            "pert": np.ascontiguousarray(perturbation[sl]),
        })
    return in_maps


def kernel(state0, W_pos, W_neg, feasibility, perturbation, t_eval=None, **kw):
    nc = _get_nc()
    in_maps = make_in_maps(state0, W_pos, W_neg, feasibility, perturbation)
    res = run_bass_kernel_spmd(nc, in_maps, core_ids=list(range(NCORES)),
                               **kw)
    out = np.concatenate([res.results[c]["out"] for c in range(NCORES)],
                         axis=1)
    if kw:
        return out, res
    return out


if __name__ == "__main__":
    inputs = {
        "state0": np.random.rand(BATCH, N).astype(np.float32),
        "W_pos": (0.2 * np.random.rand(BATCH, N, N)).astype(np.float32),
        "W_neg": (0.2 * np.random.rand(BATCH, N, N)).astype(np.float32),
        "feasibility": np.random.rand(BATCH, 4).astype(np.float32),
        "perturbation": (0.1 * np.random.randn(BATCH, N)).astype(np.float32),
    }
    out = kernel(**inputs)
    print("out", out.shape, out.dtype)



# revision 3
# speedup vs baseline: 18.2230x; 18.2230x over previous
"""Trainium2 Bass kernel for the DeepHOTCO Grossberg shunting ODE problem.

Strategy: pure data parallel over the agent batch (32768 agents -> 8 cores x
4096). Each core holds its agents as [128 partitions x 32 groups]; the
per-agent 17x17 matvecs run on the Vector engine as a broadcast multiply
followed by a grouped reduce; the elementwise shunting/Euler update is a
fused chain of DVE ops with sigmoids on the Scalar engine. The 127-step loop
is fully unrolled (no Tile back-edge barriers).
"""

import os
import sys


def _ensure_env():
    for p in (
        "/root/.axon_site",
        "/root/.axon_site/_ro/trn_rl_repo",
        "/root/.axon_site/_ro/pypackages",
        "/opt/trn_rl_repo",
    ):
        if os.path.isdir(p) and p not in sys.path:
            sys.path.append(p)
    # If the sitecustomize-driven axon boot never ran (e.g. PYTHONPATH was
    # not set for this process), replicate it. sitecustomize imports
    # trn_agent_boot, so its presence in sys.modules means boot already ran.
    if "trn_agent_boot.trn_boot" not in sys.modules and "jax" not in sys.modules:
        os.environ.setdefault("TRN_TERMINAL_POOL_IPS", "local")
        os.environ.setdefault("AXON_POOL_SVC_OVERRIDE", "127.0.0.1")
        os.environ.setdefault("AXON_LOOPBACK_RELAY", "1")
        try:
            from trn_agent_boot.trn_boot import boot

            boot(
                os.environ.get(
                    "TRN_TERMINAL_PRECOMPUTED_JSON",
                    "/root/.axon_site/_trn_precomputed.json",
                ),
                "/opt/axon/libaxon_pjrt.so",
            )
        except Exception:
            pass


_ensure_env()

import numpy as np  # noqa: E402

import concourse.bass as bass  # noqa: E402
import concourse.tile as tile  # noqa: E402
from concourse import mybir  # noqa: E402
from concourse.bass_utils import run_bass_kernel_spmd  # noqa: E402
from concourse.vector_clock import ScopedClock  # noqa: E402

# ---------------------------------------------------------------------------
# Workaround: walrus in this container only accepts a single sync-wait on the
# CTRL(Drain) instruction Tile emits at kernel tail. Split the accumulated
# waits across a chain of single-wait drains.
_MAX_DRAIN_WAITS = 1


def _patched_drain_and_barrier(self, tick_clock, wait_clock):
    drain_inst = self.nc.sync.drain()
    wait_clock.add_sem_waits(
        drain_inst.ins, ScopedClock({None: tick_clock.global_clock})
    )
    si = drain_inst.ins.sync_info
    if si is not None and si.on_wait and len(si.on_wait) > _MAX_DRAIN_WAITS:
        waits = list(si.on_wait)
        si.on_wait.clear()
        si.on_wait.extend(waits[:_MAX_DRAIN_WAITS])
        rest = waits[_MAX_DRAIN_WAITS:]
        for i in range(0, len(rest), _MAX_DRAIN_WAITS):
            extra = self.nc.sync.drain()
            esi = extra.ins.sync_info
            if esi is None:
                extra.ins.sync_info = mybir.SyncInfo(on_wait=[], on_update=[])
                esi = extra.ins.sync_info
            esi.on_wait.extend(rest[i : i + _MAX_DRAIN_WAITS])
    self.nc.all_engine_barrier()
    assert self.sems is not None
    popped = self.nc._tile_sem_poison_stack.pop()
    assert popped is self._sem_poison
    self.nc.clear_and_free_semaphores(list(self.sems.allocated().values()))
    self.nc.all_engine_barrier()


tile.TileContext._drain_and_barrier = _patched_drain_and_barrier

_waitsplit_counter = [0]


def _split_excess_waits(nc, max_waits=_MAX_DRAIN_WAITS):
    """This container's walrus accepts only one sync-wait per instruction;
    move excess waits onto same-engine NOPs inserted just before."""
    for func in nc.m.functions:
        for bb in func.blocks:
            insts = list(bb.instructions)
            needs = any(
                getattr(i, "sync_info", None) is not None
                and i.sync_info.on_wait
                and len(i.sync_info.on_wait) > max_waits
                for i in insts
            )
            if not needs:
                continue
            new_list = []
            for inst in insts:
                si = getattr(inst, "sync_info", None)
                if si is not None and si.on_wait and len(si.on_wait) > max_waits:
                    waits = list(si.on_wait)
                    del si.on_wait[max_waits:]
                    rest = waits[max_waits:]
                    for k in range(0, len(rest), max_waits):
                        _waitsplit_counter[0] += 1
                        nop = mybir.InstNoOp(
                            name=f"I-waitsplit-{_waitsplit_counter[0]}",
                            engine=inst.engine,
                            sync_info=mybir.SyncInfo(
                                on_wait=list(rest[k : k + max_waits]),
                                on_update=[],
                            ),
                        )
                        nc.register_instruction(nop)
                        new_list.append(nop)
                new_list.append(inst)
            bb.instructions[:] = new_list

# ---------------------------------------------------------------------------
# Problem constants (hardcoded per spec.json)
NCORES = 8
BATCH = 32768
P = 128          # SBUF partitions = agents per partition-block
G = 32           # agent groups along the free dim (P * G = agents per core)
N = 17           # nodes per agent
T = 128          # trajectory length (126 Euler steps + initial state)
NSTEP = T - 1
BLOC = P * G     # agents per core
GP = 7           # agent groups routed to the GpSimd (Pool) engine

TAU = 0.8
DECAY = 0.15
C_FLOOR = 0.1
LAT_INHIB = 3.0
DIV_SIGMA = 0.3
ALPHA = 1.5
BETA = 0.75
DT = 0.05
DT_TAU = DT / TAU  # 0.0625 exactly

F32 = mybir.dt.float32
AX = mybir.AxisListType
OP = mybir.AluOpType
ACTF = mybir.ActivationFunctionType


BF16 = mybir.dt.bfloat16
F16 = mybir.dt.float16
M = 2 * N  # 34 rows: [0:17] = W_pos output, [17:34] = W_neg output


def build_program(n_steps=NSTEP):
    """Combined fp16 W (pos||neg, feasibility pre-folded into pos action
    rows on host), 2x-mode fp16 broadcast multiply, pairwise-fold tree
    reduction (2x TT adds) instead of the 1x-only tensor_reduce."""
    nc = bass.Bass("TRN2", target_bir_lowering=False, debug=False,
                   num_devices=NCORES)

    x_state = nc.dram_tensor("state0", [BLOC, N], F32, kind="ExternalInput")
    # host-packed: wmain = W[:, :, 0:16] fp16, w16 = W[:, :, 16] fp16
    x_wm = nc.dram_tensor("wmain", [BLOC, M, 16], F16, kind="ExternalInput")
    x_w16 = nc.dram_tensor("w16", [BLOC, M], F16, kind="ExternalInput")
    x_pe = nc.dram_tensor("pert", [BLOC, N], F32, kind="ExternalInput")
    y = nc.dram_tensor("out", [n_steps + 1, BLOC, N], F32,
                       kind="ExternalOutput")

    from contextlib import ExitStack

    with tile.TileContext(nc) as tc, ExitStack() as ctx:
        consts = ctx.enter_context(tc.tile_pool(name="consts", bufs=1))
        states = ctx.enter_context(tc.tile_pool(name="states", bufs=3))
        tmps = ctx.enter_context(tc.tile_pool(name="tmps", bufs=1))
        folds = ctx.enter_context(tc.tile_pool(name="folds", bufs=1))
        small = ctx.enter_context(tc.tile_pool(name="small", bufs=2))

        # ---- constant loads. Small inputs first so the pre-matvec chain
        # of step 0 can run while the big W stream is still in flight.
        pe = consts.tile([P, G, N], F32, tag="pe")
        nc.sync.dma_start(out=pe, in_=x_pe[:].rearrange("(p g) n -> p g n", p=P))
        cur = states.tile([P, G, N], F32, tag="state")
        nc.sync.dma_start(out=cur,
                          in_=x_state[:].rearrange("(p g) n -> p g n", p=P))

        wm = consts.tile([P, G, M, 16], F16, tag="wm")
        wsrc = x_wm[:].rearrange("(p g) m j -> p g m j", p=P)
        nchunk = 8
        step = G // nchunk
        for c in range(nchunk):
            sl = slice(c * step, (c + 1) * step)
            nc.sync.dma_start(out=wm[:, sl], in_=wsrc[:, sl])
        w16 = consts.tile([P, G, M], F16, tag="w16")
        nc.sync.dma_start(out=w16,
                          in_=x_w16[:].rearrange("(p g) m -> p g m", p=P))

        # relu(P[:, :9]) and relu(-P[:, :9]) are loop constants (fp16)
        pp9 = consts.tile([P, G, 9], F16, tag="pp9")
        nc.vector.tensor_scalar_max(out=pp9, in0=pe[:, :, 0:9], scalar1=0.0)
        pm9 = consts.tile([P, G, 9], F16, tag="pm9")
        nc.vector.tensor_scalar(out=pm9, in0=pe[:, :, 0:9], scalar1=-1.0,
                                scalar2=0.0, op0=OP.mult, op1=OP.max)

        # trajectory row 0 = initial state
        nc.sync.dma_start(out=y[:][0].rearrange("(p g) n -> p g n", p=P),
                          in_=cur)
        sbf = states.tile([P, G, 20], F16, tag="sbf")
        nc.vector.tensor_copy(out=sbf[:, :, 0:N], in_=cur)
        nc.vector.tensor_copy(out=sbf[:, :, N:N + 1], in_=cur[:, :, 16:17])

        # ---- Euler loop (fully unrolled) -----------------------------------
        for t in range(n_steps):
            # valence gates + lateral inhibition depend only on cur; emit
            # them first so ScalarE round-trips hide under the matvec.
            ve = small.tile([P, G, 4], F32, tag="ve")
            nc.vector.tensor_add(out=ve, in0=cur[:, :, 13:17],
                                 in1=pe[:, :, 13:17])
            ge = small.tile([P, G, 4], F16, tag="ge")
            nc.scalar.activation(out=ge, in_=ve, func=ACTF.Sigmoid, scale=ALPHA)
            gi = small.tile([P, G, 4], F16, tag="gi")
            nc.scalar.activation(out=gi, in_=ve, func=ACTF.Sigmoid, scale=-BETA)

            osum = small.tile([P, G, 1], F32, tag="osum")
            nc.vector.tensor_reduce(out=osum, in_=cur[:, :, 9:13], axis=AX.X,
                                    op=OP.add)
            osum3 = small.tile([P, G, 1], F32, tag="osum3")
            nc.scalar.activation(out=osum3, in_=osum, func=ACTF.Copy,
                                 bias=DIV_SIGMA)
            den = small.tile([P, G, 4], F32, tag="den")
            nc.vector.scalar_tensor_tensor(
                out=den, in0=cur[:, :, 9:13], scalar=-1.0,
                in1=osum3.broadcast_to([P, G, 4]), op0=OP.mult, op1=OP.add)
            rec = small.tile([P, G, 4], F32, tag="rec")
            nc.vector.reciprocal(out=rec, in_=den)
            lat = small.tile([P, G, 4], F16, tag="lat")
            nc.scalar.activation(out=lat, in_=rec, func=ACTF.Copy,
                                 scale=-DIV_SIGMA * LAT_INHIB,
                                 bias=LAT_INHIB)

            # batched per-agent matvec: fp16 products at 2x, fold tree.
            # Split along the agent-group axis between DVE and the otherwise
            # idle GpSimd (Pool) engine: Pool is ~3.8x slower per element, so
            # it takes a proportionally smaller slice and both finish
            # together.
            tmp = tmps.tile([P, G, M, 16], F16, tag="tmp")
            t16 = folds.tile([P, G, M], F16, tag="t16")
            c1 = folds.tile([P, G, M, 8], F16, tag="c1")
            c2 = folds.tile([P, G, M, 4], F16, tag="c2")
            c3 = folds.tile([P, G, M, 2], F16, tag="c3")
            c4 = folds.tile([P, G, M], F16, tag="c4")
            ei = small.tile([P, G, M], F16, tag="ei")

            for eng, gs in ((nc.gpsimd, slice(0, GP)), (nc.vector, slice(GP, G))):
                gn = gs.stop - gs.start
                eng.tensor_tensor(
                    out=tmp[:, gs], in0=wm[:, gs],
                    in1=sbf[:, gs, None, 0:16].broadcast_to([P, gn, M, 16]),
                    op=OP.mult)
                eng.tensor_tensor(
                    out=t16[:, gs].rearrange("p g (q r) -> p g q r", r=2),
                    in0=w16[:, gs].rearrange("p g (q r) -> p g q r", r=2),
                    in1=sbf[:, gs, None, 16:18].broadcast_to([P, gn, N, 2]),
                    op=OP.mult)
                eng.tensor_add(out=c1[:, gs], in0=tmp[:, gs, :, 0:8],
                               in1=tmp[:, gs, :, 8:16])
                eng.tensor_add(out=c2[:, gs], in0=c1[:, gs, :, 0:4],
                               in1=c1[:, gs, :, 4:8])
                eng.tensor_add(out=c3[:, gs], in0=c2[:, gs, :, 0:2],
                               in1=c2[:, gs, :, 2:4])
                eng.tensor_add(out=c4[:, gs], in0=c3[:, gs, :, 0],
                               in1=c3[:, gs, :, 1])
                eng.tensor_add(out=ei[:, gs], in0=c4[:, gs], in1=t16[:, gs])
            # ei rows [0:17] = W_pos' @ s (feas folded), [17:34] = W_neg @ s

            # gate action rows, then relu everything on ScalarE
            nc.vector.tensor_mul(out=ei[:, :, 9:13], in0=ei[:, :, 9:13],
                                 in1=ge)
            nc.vector.tensor_mul(out=ei[:, :, 26:30], in0=ei[:, :, 26:30],
                                 in1=gi)
            nc.vector.tensor_scalar_max(out=ei, in0=ei, scalar1=0.0)

            # environmental drive on need rows
            nc.vector.tensor_add(out=ei[:, :, 0:9], in0=ei[:, :, 0:9],
                                 in1=pp9)
            nc.vector.tensor_add(out=ei[:, :, 17:26], in0=ei[:, :, 17:26],
                                 in1=pm9)

            # lateral inhibition adds to action rows of I
            nc.vector.tensor_add(out=ei[:, :, 26:30], in0=ei[:, :, 26:30],
                                 in1=lat)

            # new = s*K1 + DT_TAU*(G2 + Pd)
            #   K1 = 1 - DT_TAU*(E + I + DECAY);  G2 = E - 0.1*I
            sei = small.tile([P, G, N], F32, tag="sei")
            nc.vector.tensor_add(out=sei, in0=ei[:, :, 0:17],
                                 in1=ei[:, :, 17:34])
            k1 = small.tile([P, G, N], F32, tag="k1")
            nc.scalar.activation(out=k1, in_=sei, func=ACTF.Copy,
                                 scale=-DT_TAU, bias=1.0 - DT_TAU * DECAY)
            g2 = small.tile([P, G, N], F32, tag="g2")
            nc.vector.scalar_tensor_tensor(out=g2, in0=ei[:, :, 17:34],
                                           scalar=-C_FLOOR,
                                           in1=ei[:, :, 0:17], op0=OP.mult,
                                           op1=OP.add)
            nc.vector.tensor_add(out=g2[:, :, 13:17], in0=g2[:, :, 13:17],
                                 in1=pe[:, :, 13:17])
            t1 = small.tile([P, G, N], F32, tag="t1")
            nc.vector.tensor_mul(out=t1, in0=cur, in1=k1)
            new = states.tile([P, G, N], F32, tag="state")
            nc.vector.scalar_tensor_tensor(out=new, in0=g2, scalar=DT_TAU,
                                           in1=t1, op0=OP.mult, op1=OP.add)
            nc.vector.tensor_scalar(out=new[:, :, 0:13], in0=new[:, :, 0:13],
                                    scalar1=0.0, scalar2=1.0, op0=OP.max,
                                    op1=OP.min)
            nc.vector.tensor_scalar(out=new[:, :, 13:17], in0=new[:, :, 13:17],
                                    scalar1=-1.0, scalar2=1.0, op0=OP.max,
                                    op1=OP.min)

            nc.sync.dma_start(
                out=y[:][t + 1].rearrange("(p g) n -> p g n", p=P), in_=new)
            cur = new
            if t + 1 < n_steps:
                sbf = states.tile([P, G, 20], F16, tag="sbf")
                nc.vector.tensor_copy(out=sbf[:, :, 0:N], in_=cur)
                nc.vector.tensor_copy(out=sbf[:, :, N:N + 1],
                                      in_=cur[:, :, 16:17])

    _split_excess_waits(nc)
    return nc


_cache = {}


def _get_nc():
    if "nc" not in _cache:
        _cache["nc"] = build_program()
    return _cache["nc"]


def make_in_maps(state0, W_pos, W_neg, feasibility, perturbation):
    import ml_dtypes

    state0 = np.asarray(state0, dtype=np.float32)
    W_pos = np.asarray(W_pos, dtype=np.float32)
    W_neg = np.asarray(W_neg, dtype=np.float32)
    feasibility = np.asarray(feasibility, dtype=np.float32)
    perturbation = np.asarray(perturbation, dtype=np.float32)

    # fold feasibility into W_pos action rows (relu(F*x) == F*relu(x), F>=0)
    Wp = W_pos.copy()
    Wp[:, 9:13, :] *= feasibility[:, :, None]
    Wc = np.concatenate([Wp, W_neg], axis=1)  # [B, 34, 17]
    wmain = np.ascontiguousarray(Wc[:, :, 0:16]).astype(np.float16)
    w16 = np.ascontiguousarray(Wc[:, :, 16]).astype(np.float16)

    in_maps = []
    for c in range(NCORES):
        sl = slice(c * BLOC, (c + 1) * BLOC)
        in_maps.append({
            "state0": np.ascontiguousarray(state0[sl]),
            "wmain": np.ascontiguousarray(wmain[sl]),
            "w16": np.ascontiguousarray(w16[sl]),
            "pert": np.ascontiguousarray(perturbation[sl]),
        })
    return in_maps


def kernel(state0, W_pos, W_neg, feasibility, perturbation, t_eval=None, **kw):
    nc = _get_nc()
    in_maps = make_in_maps(state0, W_pos, W_neg, feasibility, perturbation)
    res = run_bass_kernel_spmd(nc, in_maps, core_ids=list(range(NCORES)),
                               **kw)
    out = np.concatenate([res.results[c]["out"] for c in range(NCORES)],
                         axis=1)
    if kw:
        return out, res
    return out


if __name__ == "__main__":
    inputs = {
        "state0": np.random.rand(BATCH, N).astype(np.float32),
        "W_pos": (0.2 * np.random.rand(BATCH, N, N)).astype(np.float32),
        "W_neg": (0.2 * np.random.rand(BATCH, N, N)).astype(np.float32),
        "feasibility": np.random.rand(BATCH, 4).astype(np.float32),
        "perturbation": (0.1 * np.random.randn(BATCH, N)).astype(np.float32),
    }
    out = kernel(**inputs)
    print("out", out.shape, out.dtype)

